# revision 10
# baseline (speedup 1.0000x reference)
"""DGCNN classification kernel for 8x Trainium2 NeuronCores.

Data-parallel: one point cloud (N=1024 points) per core, 8 clouds total.

Algorithmic restructuring vs the reference (numerically equivalent):
  EdgeConv: max_k LeakyReLU(BN(W @ [h_j; h_i]))
          = LeakyReLU( max_{j in knn(i)} (A1 @ h_j)  +  (A2 @ h_i + c) )
  where A = diag(bn_scale) @ W (split A1|A2 over the nbr/ctr halves) and
  c folds the BN shift; LeakyReLU commutes with max (monotone).
  This removes the [N,k,2C] edge tensor entirely: per layer we need
    - pairwise scores  s = h h^T - 0.5*||h_j||^2   (ranking-equiv to -dist^2)
    - top-20 per row (DVE max8/max_index/match_replace, 3 rounds)
    - U = h @ A1^T rows gathered by index (SWDGE dma_gather) + max over k
    - V = h @ A2^T + c, out = leaky(maxU + V)
  The trailing l3/l4/l5 affine chain is collapsed on the host into one
  [40 x 256] affine.

dma_gather wants indices wrapped [128, n/16] int16 (each Q7 core reads its
own 16 partitions; flat order J[s*16+p] with J[t*128+i] = knn[i][t] so the
gathered row for (i, t) lands at out[i, t, :]). The wrapped layout is built
on-chip with PE transposes + a replication matmul.
"""

import os
import sys

import numpy as np

sys.path.insert(0, "/opt/trn_rl_repo")

from contextlib import ExitStack  # noqa: E402

import concourse.bacc as bacc  # noqa: E402
import concourse.mybir as mybir  # noqa: E402
import concourse.tile as tile  # noqa: E402
from concourse.bass_utils import run_bass_kernel_spmd  # noqa: E402

F32 = mybir.dt.float32
U16 = mybir.dt.uint16
I16 = mybir.dt.int16
AF = mybir.ActivationFunctionType
ALU = mybir.AluOpType
AX = mybir.AxisListType

N = 1024
K = 20
B = 8
EPS = 1e-5
NEG = -3.0e38
LAYERS = [(3, 64), (64, 64), (64, 128), (128, 256)]  # (C_in, C_out)
NT = N // 128  # 8 m-tiles


# ----------------------------------------------------------------- host math
def _fold_bn(w, bn):
    g, b, m, v = [np.asarray(x, np.float32) for x in bn]
    s = (g * (1.0 / np.sqrt(v + EPS))).astype(np.float32)
    A = (s[:, None] * np.asarray(w, np.float32)).astype(np.float32)
    c = (b - m * s).astype(np.float32)
    return A, c


def _build_consts(inp):
    """All per-problem constants, shared by every core. Returns name->array."""
    c = {}
    for li, wkey, bkey in [(0, "w1", "bn1"), (1, "w2", "bn2"),
                           (2, "w3", "bn3"), (3, "w4", "bn4")]:
        Cin, Cout = LAYERS[li]
        A, cc = _fold_bn(inp[wkey], inp[bkey])
        A1, A2 = A[:, :Cin], A[:, Cin:]
        c[f"a1t{li}"] = np.ascontiguousarray(A1.T)            # [Cin, Cout]
        c[f"a2t{li}"] = np.ascontiguousarray(A2.T)            # [Cin, Cout]
        c[f"crow{li}"] = cc.reshape(1, Cout).copy()           # [1, Cout]
    # conv5
    A5, c5 = _fold_bn(inp["w5"], inp["bn5"])                  # [128, 512]
    ofs = [0, 64, 128, 256, 384, 512]
    for j in range(5):
        c[f"a5t{j}"] = np.ascontiguousarray(A5[:, ofs[j]:ofs[j + 1]].T)
    c["c5col"] = c5.reshape(128, 1).copy()
    # classifier layer 1 (512 <- 256), BN6 + leaky
    A6, c6 = _fold_bn(inp["l1w"], inp["bn6"])                 # [512, 256]
    c["a6at"] = np.ascontiguousarray(A6[:, :128].T)           # [128, 512]
    c["a6bt"] = np.ascontiguousarray((A6[:, 128:] / 1024.0).T)  # [128, 512]
    c["c6"] = np.ascontiguousarray(c6.reshape(4, 128).T)      # [128, 4]
    # classifier layer 2 (256 <- 512), +l2b then BN7 + leaky
    A7, c7 = _fold_bn(inp["l2w"], inp["bn7"])                 # [256, 512]
    s7 = np.asarray(inp["bn7"], np.float32)
    gs7 = (s7[0] * (1.0 / np.sqrt(s7[3] + EPS))).astype(np.float32)
    c7 = (c7 + gs7 * np.asarray(inp["l2b"], np.float32)).astype(np.float32)
    # lhsT layout: a7t[p, kc, m] = A7[m, kc*128+p]
    c["a7t"] = np.ascontiguousarray(A7.T.reshape(4, 128, 256).transpose(1, 0, 2))
    c["c7"] = np.ascontiguousarray(c7.reshape(2, 128).T)      # [128, 2]
    # collapse l3/l4/l5 into one affine [40, 256]
    l3w = np.asarray(inp["l3w"], np.float32); l3b = np.asarray(inp["l3b"], np.float32)
    l4w = np.asarray(inp["l4w"], np.float32); l4b = np.asarray(inp["l4b"], np.float32)
    l5w = np.asarray(inp["l5w"], np.float32); l5b = np.asarray(inp["l5b"], np.float32)
    Wc = (l5w @ l4w @ l3w).astype(np.float32)                 # [40, 256]
    bc = (l5w @ (l4w @ l3b + l4b) + l5b).astype(np.float32)   # [40]
    c["wct"] = np.ascontiguousarray(Wc.T.reshape(2, 128, 40).transpose(1, 0, 2))
    c["cout"] = bc.reshape(40, 1).copy()                      # [40, 1]
    c["ident"] = np.eye(128, dtype=np.float32)
    idrep = np.zeros((16, 128), np.float32)                   # replicate %16
    idrep[np.arange(128) % 16, np.arange(128)] = 1.0
    c["idrep"] = idrep
    return c


# --------------------------------------------------------------- the program
def _emit(tc, io):
    """Emit the full per-core program. io: name -> DRAM AP."""
    nc = tc.nc
    ctx = ExitStack()

    cp = ctx.enter_context(tc.tile_pool(name="consts", bufs=1))
    hp = ctx.enter_context(tc.tile_pool(name="hbufs", bufs=1))
    sp = ctx.enter_context(tc.tile_pool(name="swork", bufs=3))
    wp = ctx.enter_context(tc.tile_pool(name="work", bufs=2))
    ip = ctx.enter_context(tc.tile_pool(name="idxp", bufs=2))
    vp = ctx.enter_context(tc.tile_pool(name="needles", bufs=3))
    nb = ctx.enter_context(tc.tile_pool(name="nbrp", bufs=2))
    dp = ctx.enter_context(tc.tile_pool(name="dramp", bufs=1, space="DRAM"))
    ps_s = ctx.enter_context(tc.tile_pool(name="ps_s", bufs=2, space="PSUM"))
    ps_v = ctx.enter_context(tc.tile_pool(name="ps_v", bufs=2, space="PSUM"))
    ps_a = ctx.enter_context(tc.tile_pool(name="ps_a", bufs=2, space="PSUM"))
    ps_r = ctx.enter_context(tc.tile_pool(name="ps_r", bufs=2, space="PSUM"))

    def load_const(name):
        shp = list(io[name].shape)
        t = cp.tile(shp, F32, name=f"c_{name}", tag=f"c_{name}")
        nc.sync.dma_start(t[...], io[name])
        return t

    consts = {k: load_const(k) for k in io if k not in ("hx", "out")}
    ones_row = cp.tile([1, 128], F32, name="ones_row", tag="ones_row")
    nc.gpsimd.memset(ones_row[:], 1.0)
    neghalf = cp.tile([128, 1], F32, name="neghalf", tag="neghalf")
    nc.gpsimd.memset(neghalf[:], -0.5)
    ident = consts["ident"]
    idrep = consts["idrep"]

    # h^T buffers, feature-major [C, N]
    hxT = hp.tile([3, N], F32, name="hxT", tag="hxT")
    nc.sync.dma_start(hxT[...], io["hx"])
    h1T = hp.tile([64, N], F32, name="h1T", tag="h1T")
    h2T = hp.tile([64, N], F32, name="h2T", tag="h2T")
    h3T = hp.tile([128, N], F32, name="h3T", tag="h3T")
    h4Ta = hp.tile([128, N], F32, name="h4Ta", tag="h4Ta")
    h4Tb = hp.tile([128, N], F32, name="h4Tb", tag="h4Tb")
    h5T = hp.tile([128, N], F32, name="h5T", tag="h5T")

    h_in = [hxT, h1T, h2T, h3T]
    h_out = [[h1T], [h2T], [h3T], [h4Ta, h4Tb]]

    # ------------------------------------------------------------ edge convs
    for li, (Cin, Cout) in enumerate(LAYERS):
        hT = h_in[li]
        a1t = consts[f"a1t{li}"]
        a2t = consts[f"a2t{li}"]
        crow = consts[f"crow{li}"]
        nch = (Cout + 127) // 128  # channel halves (1 or 2)

        # squared norms: xx[n] = -0.5 * sum_c h[c,n]^2, via Square + ones-matmul
        sq = wp.tile([Cin, N], F32, name=f"sq{li}", tag="sq")
        nc.scalar.activation(sq[...], hT[...], AF.Square)
        xx_sb = wp.tile([1, N], F32, name=f"xx{li}", tag="xx")
        for nt2 in range(2):
            xx_ps = ps_a.tile([1, 512], F32, name=f"xxps{li}_{nt2}", tag="aux")
            nc.tensor.matmul(xx_ps[...], neghalf[0:Cin, :],
                             sq[:, nt2 * 512:(nt2 + 1) * 512], start=True, stop=True)
            nc.scalar.copy(xx_sb[:, nt2 * 512:(nt2 + 1) * 512], xx_ps[...])

        # ---------------- phase A per m-tile: scores, topk, U
        idx_all = ip.tile([128, NT, 24], U16, name=f"idxall{li}", tag="idxall")
        for m in range(NT):
            mb = slice(m * 128, (m + 1) * 128)
            s_sb = sp.tile([128, N], F32, name=f"s{li}_{m}", tag="s_sb")
            for nt2 in range(2):
                ns = slice(nt2 * 512, (nt2 + 1) * 512)
                s_ps = ps_s.tile([128, 512], F32, name=f"sps{li}_{m}_{nt2}", tag="sps")
                nc.tensor.matmul(s_ps[...], hT[:, mb], hT[:, ns],
                                 start=True, stop=False)
                nc.tensor.matmul(s_ps[...], ones_row[...], xx_sb[:, ns],
                                 start=False, stop=True)
                nc.scalar.copy(s_sb[:, ns], s_ps[...])

            # top-20 of each row: 3 rounds of (max8, max_index) + match_replace
            for r in range(3):
                v8 = vp.tile([128, 8], F32, name=f"v8_{li}_{m}_{r}", tag="v8")
                nc.vector.max(v8[...], s_sb[...])
                nc.vector.max_index(idx_all[:, m, r * 8:(r + 1) * 8], v8[...],
                                    s_sb[...])
                if r < 2:
                    nc.vector.match_replace(s_sb[...], v8[...], s_sb[...], NEG)


        # ---------------- U^T feature-major [Cout, N] in SBUF
        uts = []
        for ci in range(nch):
            cs = slice(ci * 128, min((ci + 1) * 128, Cout))
            w = cs.stop - cs.start
            ut = wp.tile([128, N], F32, name=f"ut{li}_{ci}", tag=f"ut{ci}")
            for nt2 in range(2):
                ns = slice(nt2 * 512, (nt2 + 1) * 512)
                ut_ps = ps_s.tile([128, 512], F32, name=f"utps{li}_{ci}_{nt2}",
                                  tag="sps")
                nc.tensor.matmul(ut_ps[0:w, :], a1t[:, cs], hT[:, ns],
                                 start=True, stop=True)
                nc.scalar.copy(ut[0:w, ns], ut_ps[0:w, :])
            uts.append(ut)

        # ---------------- batched idx-layout build (one u16->f32 cast/layer)
        idxf_all = wp.tile([128, NT, 20], F32, name=f"idxfall{li}", tag="idxfall")
        nc.vector.tensor_copy(idxf_all[...], idx_all[:, :, 0:20])
        idxs_all = ip.tile([128, NT, 160], I16, name=f"idxsall{li}", tag="idxsall")
        for g in range(3):             # m-tile batches (3, 3, 2)
            ms = list(range(NT))[g * 3:(g + 1) * 3]
            rep_ps = ps_r.tile([128, 480], F32, name=f"repps{li}_{g}", tag="repall")
            for j, m in enumerate(ms):
                mt_ps = ps_a.tile([20, 128], F32, name=f"mtps{li}_{m}", tag="aux")
                nc.tensor.transpose(mt_ps[...], idxf_all[:, m, :], ident[...])
                mt_sb = wp.tile([20, 128], F32, name=f"mtsb{li}_{m}", tag="mtsb")
                nc.scalar.copy(mt_sb[...], mt_ps[...])
                tball = ps_a.tile([16, 20, 8], F32, name=f"tball{li}_{m}", tag="aux")
                for bb in range(8):
                    nc.tensor.transpose(tball[:, :, bb],
                                        mt_sb[:, bb * 16:(bb + 1) * 16],
                                        ident[0:20, 0:20])
                wsb = wp.tile([16, 160], F32, name=f"wsb{li}_{m}", tag="wsb")
                nc.scalar.copy(wsb[...], tball[...].rearrange("p t b -> p (t b)"))
                nc.tensor.matmul(rep_ps[:, j * 160:(j + 1) * 160], idrep[...],
                                 wsb[...], start=True, stop=True)
            nw = len(ms) * 160
            nc.vector.tensor_copy(
                idxs_all[:, g * 3:g * 3 + len(ms), :].rearrange("p a b -> p (a b)"),
                rep_ps[:, 0:nw])

        # ---------------- phase B per m-tile: gather, k-max, V, combine
        for m in range(NT):
            mb = slice(m * 128, (m + 1) * 128)
            for ci, hdst in enumerate(h_out[li]):
                cs = slice(ci * 128, min((ci + 1) * 128, Cout))
                w = cs.stop - cs.start
                # nbrT[o, t, i] = U^T[o, knn[i][t]]  (J order t*128+i)
                nbrT = nb.tile([128, K, 128], F32, name=f"nbr{li}_{m}_{ci}",
                               tag="nbr")
                nc.gpsimd.ap_gather(
                    out_ap=nbrT[0:w, :, :].rearrange("o t i -> o (t i)")
                                          .unsqueeze(-1),
                    in_ap=uts[ci][0:w, :].unsqueeze(-1),
                    idxs_ap=idxs_all[0:w, m, :],
                    channels=w, num_elems=N, d=1, num_idxs=128 * K)
                mxT = wp.tile([128, 128], F32, name=f"mx{li}_{m}_{ci}", tag="mx")
                # strided view [o, i, t] over the [o, t, i] buffer; reduce t
                nc.vector.tensor_reduce(
                    mxT[0:w, :], nbrT[0:w, :, :].rearrange("o t i -> o i t"),
                    axis=AX.X, op=ALU.max)

                v_ps = ps_v.tile([128, 128], F32, name=f"vps{li}_{m}_{ci}",
                                 tag="vps")
                nc.tensor.matmul(v_ps[0:w, :], a2t[:, cs], hT[:, mb],
                                 start=True, stop=False)
                nc.tensor.matmul(v_ps[0:w, :], crow[:, cs], ones_row[...],
                                 start=False, stop=True)

                z = wp.tile([128, 128], F32, name=f"z{li}_{m}_{ci}", tag="z")
                nc.vector.tensor_tensor(z[0:w, :], mxT[0:w, :], v_ps[0:w, :],
                                        ALU.add)
                nc.vector.scalar_tensor_tensor(hdst[0:w, mb], z[0:w, :], 0.2,
                                               z[0:w, :], op0=ALU.mult,
                                               op1=ALU.max)

    # ------------------------------------------------------------ conv5
    a5 = [consts[f"a5t{j}"] for j in range(5)]
    srcs = [h1T, h2T, h3T, h4Ta, h4Tb]
    for nt2 in range(2):
        ns = slice(nt2 * 512, (nt2 + 1) * 512)
        h5_ps = ps_s.tile([128, 512], F32, name=f"h5ps{nt2}", tag="sps")
        for j in range(5):
            nc.tensor.matmul(h5_ps[...], a5[j][...], srcs[j][:, ns],
                             start=(j == 0), stop=(j == 4))
        # h5 = leaky(h5_ps + c5); c5 is per-channel = per-partition scalar bias
        zt = sp.tile([128, 512], F32, name=f"h5z{nt2}", tag="s_sb")
        nc.scalar.activation(zt[:, 0:512], h5_ps[...], AF.Identity,
                             bias=consts["c5col"][...])
        nc.vector.scalar_tensor_tensor(h5T[:, ns], zt[:, 0:512], 0.2,
                                       zt[:, 0:512], op0=ALU.mult, op1=ALU.max)

    # ------------------------------------------------------------ pooling
    gmax = wp.tile([128, 1], F32, name="gmax", tag="gpool")
    nc.vector.tensor_reduce(gmax[...], h5T[...], axis=AX.X, op=ALU.max)
    gsum = wp.tile([128, 1], F32, name="gsum", tag="gpool")
    nc.vector.tensor_reduce(gsum[...], h5T[...], axis=AX.X, op=ALU.add)

    # ------------------------------------------------------------ classifier
    a6at, a6bt, c6 = consts["a6at"], consts["a6bt"], consts["c6"]
    y1 = wp.tile([128, 4], F32, name="y1", tag="y1")
    for mt in range(4):
        ms = slice(mt * 128, (mt + 1) * 128)
        y_ps = ps_v.tile([128, 1], F32, name=f"y1ps{mt}", tag="vps")
        nc.tensor.matmul(y_ps[...], a6at[:, ms], gmax[...], start=True, stop=False)
        nc.tensor.matmul(y_ps[...], a6bt[:, ms], gsum[...], start=False, stop=True)
        nc.scalar.activation(y1[:, mt:mt + 1], y_ps[...], AF.Identity,
                             bias=c6[:, mt:mt + 1])
    y1l = wp.tile([128, 4], F32, name="y1l", tag="y1")
    nc.vector.scalar_tensor_tensor(y1l[...], y1[...], 0.2, y1[...],
                                   op0=ALU.mult, op1=ALU.max)

    a7t, c7 = consts["a7t"], consts["c7"]
    y2 = wp.tile([128, 2], F32, name="y2", tag="y2")
    for m2 in range(2):
        ms = slice(m2 * 128, (m2 + 1) * 128)
        y_ps = ps_v.tile([128, 1], F32, name=f"y2ps{m2}", tag="vps")
        for kc in range(4):
            nc.tensor.matmul(y_ps[...], a7t[:, kc, ms], y1l[:, kc:kc + 1],
                             start=(kc == 0), stop=(kc == 3))
        nc.scalar.activation(y2[:, m2:m2 + 1], y_ps[...], AF.Identity,
                             bias=c7[:, m2:m2 + 1])
    y2l = wp.tile([128, 2], F32, name="y2l", tag="y2")
    nc.vector.scalar_tensor_tensor(y2l[...], y2[...], 0.2, y2[...],
                                   op0=ALU.mult, op1=ALU.max)

    wct, cout = consts["wct"], consts["cout"]
    y5_ps = ps_v.tile([40, 1], F32, name="y5ps", tag="vps")
    for kc in range(2):
        nc.tensor.matmul(y5_ps[...], wct[:, kc, :], y2l[:, kc:kc + 1],
                         start=(kc == 0), stop=(kc == 1))
    y5 = wp.tile([40, 1], F32, name="y5", tag="y5")
    nc.scalar.activation(y5[...], y5_ps[...], AF.Identity, bias=cout[...])
    nc.sync.dma_start(io["out"], y5[...])

    ctx.close()


def _install_profile_hook():
    """The agent image's antenv lacks axon_hooks; recreate it so trace=True
    can drive NTFF profiling through libaxon_pjrt.so (test-only path)."""
    import types
    try:
        from antenv.axon_hooks import get_axon_ntff_profile_hook  # noqa: F401
        return
    except ImportError:
        pass
    mod = types.ModuleType("antenv.axon_hooks")
    _h = [None]
    mod.set_axon_ntff_profile_hook = lambda h: _h.__setitem__(0, h)
    mod.get_axon_ntff_profile_hook = lambda: _h[0]
    import antenv
    antenv.axon_hooks = mod
    sys.modules["antenv.axon_hooks"] = mod
    if "/root/.axon_site" not in sys.path:
        sys.path.insert(0, "/root/.axon_site")
    from trn_agent_boot.trn_boot import _ntff_profile_via_ctypes
    mod.set_axon_ntff_profile_hook(
        _ntff_profile_via_ctypes("/opt/axon/libaxon_pjrt.so"))
    # artifact upload needs the fish bucket; stub it out in-container
    import concourse.bass_utils as _bu
    _bu.upload_artifacts = lambda tmpdir: tmpdir


# --------------------------------------------------------------- build + run
_CACHE = {}


def _build_program(const_shapes):
    nc = bacc.Bacc("TRN2", target_bir_lowering=False, debug=False,
                   enable_asserts=False, num_devices=B)
    io = {}
    io["hx"] = nc.dram_tensor("hx", [3, N], F32, kind="ExternalInput").ap()
    for name, shp in const_shapes.items():
        io[name] = nc.dram_tensor(name, list(shp), F32, kind="ExternalInput").ap()
    io["out"] = nc.dram_tensor("out", [40], F32, kind="ExternalOutput").ap()
    with tile.TileContext(nc) as tc:
        _emit(tc, io)
    nc.compile()
    return nc


def kernel(**inputs):
    consts = _build_consts(inputs)
    key = "prog"
    if key not in _CACHE:
        _CACHE[key] = _build_program({k: v.shape for k, v in consts.items()})
    nc = _CACHE[key]

    x = np.asarray(inputs["x"], np.float32)
    in_maps = []
    for bi in range(B):
        m = {"hx": np.ascontiguousarray(x[bi])}
        m.update(consts)
        in_maps.append(m)

    trace = bool(int(os.environ.get("KERNEL_TRACE", "0")))
    if trace:
        _install_profile_hook()
    res = run_bass_kernel_spmd(nc, in_maps, core_ids=list(range(B)), trace=trace)
    kernel.last_result = res
    out = np.stack([r["out"] for r in res.results], axis=0).astype(np.float32)
    return out


if __name__ == "__main__":
    import reference as R
    inp = {k: np.asarray(v) for k, v in R.setup_inputs().items()}
    got = kernel(**inp)
    exp = np.asarray(R.reference(**R.setup_inputs()))
    err = np.abs(got - exp).max() / np.abs(exp).max()
    print("rel err:", err)


# revision 12
# speedup vs baseline: 2.8644x; 2.8644x over previous
"""DGCNN classification kernel for 8x Trainium2 NeuronCores.

Data-parallel: one point cloud (N=1024 points) per core, 8 clouds total.

Algorithmic restructuring vs the reference (numerically equivalent):
  EdgeConv: max_k LeakyReLU(BN(W @ [h_j; h_i]))
          = LeakyReLU( max_{j in knn(i)} (A1 @ h_j)  +  (A2 @ h_i + c) )
  where A = diag(bn_scale) @ W (split A1|A2 over the nbr/ctr halves) and
  c folds the BN shift; LeakyReLU commutes with max (monotone).
  This removes the [N,k,2C] edge tensor entirely: per layer we need
    - pairwise scores  s = h h^T - 0.5*||h_j||^2   (ranking-equiv to -dist^2)
    - top-20 per row (DVE max8/max_index/match_replace, 3 rounds)
    - U = h @ A1^T rows gathered by index (SWDGE dma_gather) + max over k
    - V = h @ A2^T + c, out = leaky(maxU + V)
  The trailing l3/l4/l5 affine chain is collapsed on the host into one
  [40 x 256] affine.

dma_gather wants indices wrapped [128, n/16] int16 (each Q7 core reads its
own 16 partitions; flat order J[s*16+p] with J[t*128+i] = knn[i][t] so the
gathered row for (i, t) lands at out[i, t, :]). The wrapped layout is built
on-chip with PE transposes + a replication matmul.
"""

import os
import sys

import numpy as np

sys.path.insert(0, "/opt/trn_rl_repo")

from contextlib import ExitStack  # noqa: E402

import concourse.bacc as bacc  # noqa: E402
import concourse.mybir as mybir  # noqa: E402
import concourse.tile as tile  # noqa: E402
from concourse.bass_utils import run_bass_kernel_spmd  # noqa: E402

F32 = mybir.dt.float32
U16 = mybir.dt.uint16
I16 = mybir.dt.int16
AF = mybir.ActivationFunctionType
ALU = mybir.AluOpType
AX = mybir.AxisListType

N = 1024
K = 20
B = 8
EPS = 1e-5
NEG = -3.0e38
LAYERS = [(3, 64), (64, 64), (64, 128), (128, 256)]  # (C_in, C_out)
NT = N // 128  # 8 m-tiles


# ----------------------------------------------------------------- host math
def _fold_bn(w, bn):
    g, b, m, v = [np.asarray(x, np.float32) for x in bn]
    s = (g * (1.0 / np.sqrt(v + EPS))).astype(np.float32)
    A = (s[:, None] * np.asarray(w, np.float32)).astype(np.float32)
    c = (b - m * s).astype(np.float32)
    return A, c


def _build_consts(inp):
    """All per-problem constants, shared by every core. Returns name->array."""
    c = {}
    for li, wkey, bkey in [(0, "w1", "bn1"), (1, "w2", "bn2"),
                           (2, "w3", "bn3"), (3, "w4", "bn4")]:
        Cin, Cout = LAYERS[li]
        A, cc = _fold_bn(inp[wkey], inp[bkey])
        A1, A2 = A[:, :Cin], A[:, Cin:]
        c[f"a1t{li}"] = np.ascontiguousarray(A1.T)            # [Cin, Cout]
        c[f"a2t{li}"] = np.ascontiguousarray(A2.T)            # [Cin, Cout]
        c[f"crow{li}"] = cc.reshape(1, Cout).copy()           # [1, Cout]
    # conv5
    A5, c5 = _fold_bn(inp["w5"], inp["bn5"])                  # [128, 512]
    ofs = [0, 64, 128, 256, 384, 512]
    for j in range(5):
        c[f"a5t{j}"] = np.ascontiguousarray(A5[:, ofs[j]:ofs[j + 1]].T)
    c["c5col"] = c5.reshape(128, 1).copy()
    # classifier layer 1 (512 <- 256), BN6 + leaky
    A6, c6 = _fold_bn(inp["l1w"], inp["bn6"])                 # [512, 256]
    c["a6at"] = np.ascontiguousarray(A6[:, :128].T)           # [128, 512]
    c["a6bt"] = np.ascontiguousarray((A6[:, 128:] / 1024.0).T)  # [128, 512]
    c["c6"] = np.ascontiguousarray(c6.reshape(4, 128).T)      # [128, 4]
    # classifier layer 2 (256 <- 512), +l2b then BN7 + leaky
    A7, c7 = _fold_bn(inp["l2w"], inp["bn7"])                 # [256, 512]
    s7 = np.asarray(inp["bn7"], np.float32)
    gs7 = (s7[0] * (1.0 / np.sqrt(s7[3] + EPS))).astype(np.float32)
    c7 = (c7 + gs7 * np.asarray(inp["l2b"], np.float32)).astype(np.float32)
    # lhsT layout: a7t[p, kc, m] = A7[m, kc*128+p]
    c["a7t"] = np.ascontiguousarray(A7.T.reshape(4, 128, 256).transpose(1, 0, 2))
    c["c7"] = np.ascontiguousarray(c7.reshape(2, 128).T)      # [128, 2]
    # collapse l3/l4/l5 into one affine [40, 256]
    l3w = np.asarray(inp["l3w"], np.float32); l3b = np.asarray(inp["l3b"], np.float32)
    l4w = np.asarray(inp["l4w"], np.float32); l4b = np.asarray(inp["l4b"], np.float32)
    l5w = np.asarray(inp["l5w"], np.float32); l5b = np.asarray(inp["l5b"], np.float32)
    Wc = (l5w @ l4w @ l3w).astype(np.float32)                 # [40, 256]
    bc = (l5w @ (l4w @ l3b + l4b) + l5b).astype(np.float32)   # [40]
    c["wct"] = np.ascontiguousarray(Wc.T.reshape(2, 128, 40).transpose(1, 0, 2))
    c["cout"] = bc.reshape(40, 1).copy()                      # [40, 1]
    c["ident"] = np.eye(128, dtype=np.float32)
    idrep = np.zeros((16, 128), np.float32)                   # replicate %16
    idrep[np.arange(128) % 16, np.arange(128)] = 1.0
    c["idrep"] = idrep
    return c


# --------------------------------------------------------------- the program
def _emit(tc, io):
    """Emit the full per-core program. io: name -> DRAM AP."""
    nc = tc.nc
    ctx = ExitStack()

    cp = ctx.enter_context(tc.tile_pool(name="consts", bufs=1))
    hp = ctx.enter_context(tc.tile_pool(name="hbufs", bufs=1))
    sp = ctx.enter_context(tc.tile_pool(name="swork", bufs=3))
    wp = ctx.enter_context(tc.tile_pool(name="work", bufs=2))
    ip = ctx.enter_context(tc.tile_pool(name="idxp", bufs=2))
    vp = ctx.enter_context(tc.tile_pool(name="needles", bufs=3))
    nb = ctx.enter_context(tc.tile_pool(name="nbrp", bufs=2))
    dp = ctx.enter_context(tc.tile_pool(name="dramp", bufs=1, space="DRAM"))
    ps_s = ctx.enter_context(tc.tile_pool(name="ps_s", bufs=2, space="PSUM"))
    ps_u = ctx.enter_context(tc.tile_pool(name="ps_u", bufs=1, space="PSUM"))
    ps_v = ctx.enter_context(tc.tile_pool(name="ps_v", bufs=2, space="PSUM"))
    ps_a = ctx.enter_context(tc.tile_pool(name="ps_a", bufs=1, space="PSUM"))
    ps_r = ctx.enter_context(tc.tile_pool(name="ps_r", bufs=2, space="PSUM"))

    def load_const(name):
        shp = list(io[name].shape)
        t = cp.tile(shp, F32, name=f"c_{name}", tag=f"c_{name}")
        nc.sync.dma_start(t[...], io[name])
        return t

    consts = {k: load_const(k) for k in io if k not in ("hx", "out")}
    ones_row = cp.tile([1, 128], F32, name="ones_row", tag="ones_row")
    nc.gpsimd.memset(ones_row[:], 1.0)
    neghalf = cp.tile([128, 1], F32, name="neghalf", tag="neghalf")
    nc.gpsimd.memset(neghalf[:], -0.5)
    ident = consts["ident"]
    idrep = consts["idrep"]

    # h^T buffers, feature-major [C, N]
    hxT = hp.tile([3, N], F32, name="hxT", tag="hxT")
    nc.sync.dma_start(hxT[...], io["hx"])
    h1T = hp.tile([64, N], F32, name="h1T", tag="h1T")
    h2T = hp.tile([64, N], F32, name="h2T", tag="h2T")
    h3T = hp.tile([128, N], F32, name="h3T", tag="h3T")
    h4Ta = hp.tile([128, N], F32, name="h4Ta", tag="h4Ta")
    h4Tb = hp.tile([128, N], F32, name="h4Tb", tag="h4Tb")
    h5T = hp.tile([128, N], F32, name="h5T", tag="h5T")

    h_in = [hxT, h1T, h2T, h3T]
    h_out = [[h1T], [h2T], [h3T], [h4Ta, h4Tb]]

    # ------------------------------------------------------------ edge convs
    for li, (Cin, Cout) in enumerate(LAYERS):
        hT = h_in[li]
        a1t = consts[f"a1t{li}"]
        a2t = consts[f"a2t{li}"]
        crow = consts[f"crow{li}"]
        u_dram = dp.tile([N, Cout], F32, name=f"u_dram{li}", tag=f"u_dram{li}")

        # squared norms: xx[n] = -0.5 * sum_c h[c,n]^2, via Square + ones-matmul
        sq = wp.tile([Cin, N], F32, name=f"sq{li}", tag="sq")
        nc.scalar.activation(sq[...], hT[...], AF.Square)
        xx_sb = wp.tile([1, N], F32, name=f"xx{li}", tag="xx")
        for nt2 in range(2):
            xx_ps = ps_a.tile([1, 512], F32, name=f"xxps{li}_{nt2}", tag="aux")
            nc.tensor.matmul(xx_ps[...], neghalf[0:Cin, :],
                             sq[:, nt2 * 512:(nt2 + 1) * 512], start=True, stop=True)
            nc.scalar.copy(xx_sb[:, nt2 * 512:(nt2 + 1) * 512], xx_ps[...])

        # ---------------- phase A per m-tile: scores, topk, U
        idx_all = ip.tile([128, NT, 24], U16, name=f"idxall{li}", tag="idxall")
        for m in range(NT):
            mb = slice(m * 128, (m + 1) * 128)
            s_sb = sp.tile([128, N], F32, name=f"s{li}_{m}", tag="s_sb")
            for nt2 in range(2):
                ns = slice(nt2 * 512, (nt2 + 1) * 512)
                s_ps = ps_s.tile([128, 512], F32, name=f"sps{li}_{m}_{nt2}", tag="sps")
                nc.tensor.matmul(s_ps[...], hT[:, mb], hT[:, ns],
                                 start=True, stop=False)
                nc.tensor.matmul(s_ps[...], ones_row[...], xx_sb[:, ns],
                                 start=False, stop=True)
                nc.scalar.copy(s_sb[:, ns], s_ps[...])

            # top-20 of each row: 3 rounds of (max8, max_index) + match_replace
            for r in range(3):
                v8 = vp.tile([128, 8], F32, name=f"v8_{li}_{m}_{r}", tag="v8")
                nc.vector.max(v8[...], s_sb[...])
                nc.vector.max_index(idx_all[:, m, r * 8:(r + 1) * 8], v8[...],
                                    s_sb[...])
                if r < 2:
                    nc.vector.match_replace(s_sb[...], v8[...], s_sb[...], NEG)

            # U tile: [128 pts, Cout] point-major, stored to DRAM for the gather
            u_ps = ps_u.tile([128, Cout], F32, name=f"ups{li}_{m}", tag="ups")
            nc.tensor.matmul(u_ps[...], hT[:, mb], a1t[...], start=True, stop=True)
            u_sb = wp.tile([128, Cout], F32, name=f"usb{li}_{m}", tag="usb")
            nc.scalar.copy(u_sb[...], u_ps[...])
            nc.sync.dma_start(u_dram[mb, :], u_sb[...])

        # ---------------- batched idx-layout build (one u16->f32 cast/layer)
        idxf_all = wp.tile([128, NT, 20], F32, name=f"idxfall{li}", tag="idxfall")
        nc.vector.tensor_copy(idxf_all[...], idx_all[:, :, 0:20])
        idxs_all = ip.tile([128, NT, 160], I16, name=f"idxsall{li}", tag="idxsall")
        for g in range(3):             # m-tile batches (3, 3, 2)
            ms = list(range(NT))[g * 3:(g + 1) * 3]
            rep_ps = ps_r.tile([128, 480], F32, name=f"repps{li}_{g}", tag="repall")
            for j, m in enumerate(ms):
                mt_ps = ps_a.tile([20, 128], F32, name=f"mtps{li}_{m}", tag="aux")
                nc.tensor.transpose(mt_ps[...], idxf_all[:, m, :], ident[...])
                mt_sb = wp.tile([20, 128], F32, name=f"mtsb{li}_{m}", tag="mtsb")
                nc.scalar.copy(mt_sb[...], mt_ps[...])
                tball = ps_a.tile([16, 20, 8], F32, name=f"tball{li}_{m}", tag="aux")
                for bb in range(8):
                    nc.tensor.transpose(tball[:, :, bb],
                                        mt_sb[:, bb * 16:(bb + 1) * 16],
                                        ident[0:20, 0:20])
                wsb = wp.tile([16, 160], F32, name=f"wsb{li}_{m}", tag="wsb")
                nc.scalar.copy(wsb[...], tball[...].rearrange("p t b -> p (t b)"))
                nc.tensor.matmul(rep_ps[:, j * 160:(j + 1) * 160], idrep[...],
                                 wsb[...], start=True, stop=True)
            nw = len(ms) * 160
            nc.vector.tensor_copy(
                idxs_all[:, g * 3:g * 3 + len(ms), :].rearrange("p a b -> p (a b)"),
                rep_ps[:, 0:nw])

        # ---------------- phase B per m-tile: gather, k-max, V, combine
        for m in range(NT):
            mb = slice(m * 128, (m + 1) * 128)
            nbr = nb.tile([128, K, Cout], F32, name=f"nbr{li}_{m}", tag="nbr")
            nc.gpsimd.dma_gather(nbr[...], u_dram[...], idxs_all[:, m, :],
                                 num_idxs=128 * K, num_idxs_reg=128 * K,
                                 elem_size=Cout, single_packet=False)
            mx = wp.tile([128, Cout], F32, name=f"mx{li}_{m}", tag="mx")
            nc.vector.tensor_reduce(mx[...], nbr[...].rearrange("p t c -> p c t"),
                                    axis=AX.X, op=ALU.max)

            v_ps = ps_v.tile([128, Cout], F32, name=f"vps{li}_{m}", tag="vps")
            nc.tensor.matmul(v_ps[...], hT[:, mb], a2t[...], start=True, stop=False)
            nc.tensor.matmul(v_ps[...], ones_row[...], crow[...],
                             start=False, stop=True)

            z = wp.tile([128, Cout], F32, name=f"z{li}_{m}", tag="z")
            nc.vector.tensor_tensor(z[...], mx[...], v_ps[...], ALU.add)
            hpm = wp.tile([128, Cout], F32, name=f"hpm{li}_{m}", tag="hpm")
            nc.vector.scalar_tensor_tensor(hpm[...], z[...], 0.2, z[...],
                                           op0=ALU.mult, op1=ALU.max)

            # transpose back to feature-major into next layer's h^T buffer
            for ci, hdst in enumerate(h_out[li]):
                cs = slice(ci * 128, min((ci + 1) * 128, Cout))
                w = cs.stop - cs.start
                t_ps = ps_a.tile([w, 128], F32, name=f"tps{li}_{m}_{ci}", tag="aux")
                nc.tensor.transpose(t_ps[...], hpm[:, cs], ident[...])
                nc.scalar.copy(hdst[0:w, mb], t_ps[...])

    # ------------------------------------------------------------ conv5
    a5 = [consts[f"a5t{j}"] for j in range(5)]
    srcs = [h1T, h2T, h3T, h4Ta, h4Tb]
    for nt2 in range(2):
        ns = slice(nt2 * 512, (nt2 + 1) * 512)
        h5_ps = ps_s.tile([128, 512], F32, name=f"h5ps{nt2}", tag="sps")
        for j in range(5):
            nc.tensor.matmul(h5_ps[...], a5[j][...], srcs[j][:, ns],
                             start=(j == 0), stop=(j == 4))
        # h5 = leaky(h5_ps + c5); c5 is per-channel = per-partition scalar bias
        zt = sp.tile([128, 512], F32, name=f"h5z{nt2}", tag="s_sb")
        nc.scalar.activation(zt[:, 0:512], h5_ps[...], AF.Identity,
                             bias=consts["c5col"][...])
        nc.vector.scalar_tensor_tensor(h5T[:, ns], zt[:, 0:512], 0.2,
                                       zt[:, 0:512], op0=ALU.mult, op1=ALU.max)

    # ------------------------------------------------------------ pooling
    gmax = wp.tile([128, 1], F32, name="gmax", tag="gpool")
    nc.vector.tensor_reduce(gmax[...], h5T[...], axis=AX.X, op=ALU.max)
    gsum = wp.tile([128, 1], F32, name="gsum", tag="gpool")
    nc.vector.tensor_reduce(gsum[...], h5T[...], axis=AX.X, op=ALU.add)

    # ------------------------------------------------------------ classifier
    a6at, a6bt, c6 = consts["a6at"], consts["a6bt"], consts["c6"]
    y1 = wp.tile([128, 4], F32, name="y1", tag="y1")
    for mt in range(4):
        ms = slice(mt * 128, (mt + 1) * 128)
        y_ps = ps_v.tile([128, 1], F32, name=f"y1ps{mt}", tag="vps")
        nc.tensor.matmul(y_ps[...], a6at[:, ms], gmax[...], start=True, stop=False)
        nc.tensor.matmul(y_ps[...], a6bt[:, ms], gsum[...], start=False, stop=True)
        nc.scalar.activation(y1[:, mt:mt + 1], y_ps[...], AF.Identity,
                             bias=c6[:, mt:mt + 1])
    y1l = wp.tile([128, 4], F32, name="y1l", tag="y1")
    nc.vector.scalar_tensor_tensor(y1l[...], y1[...], 0.2, y1[...],
                                   op0=ALU.mult, op1=ALU.max)

    a7t, c7 = consts["a7t"], consts["c7"]
    y2 = wp.tile([128, 2], F32, name="y2", tag="y2")
    for m2 in range(2):
        ms = slice(m2 * 128, (m2 + 1) * 128)
        y_ps = ps_v.tile([128, 1], F32, name=f"y2ps{m2}", tag="vps")
        for kc in range(4):
            nc.tensor.matmul(y_ps[...], a7t[:, kc, ms], y1l[:, kc:kc + 1],
                             start=(kc == 0), stop=(kc == 3))
        nc.scalar.activation(y2[:, m2:m2 + 1], y_ps[...], AF.Identity,
                             bias=c7[:, m2:m2 + 1])
    y2l = wp.tile([128, 2], F32, name="y2l", tag="y2")
    nc.vector.scalar_tensor_tensor(y2l[...], y2[...], 0.2, y2[...],
                                   op0=ALU.mult, op1=ALU.max)

    wct, cout = consts["wct"], consts["cout"]
    y5_ps = ps_v.tile([40, 1], F32, name="y5ps", tag="vps")
    for kc in range(2):
        nc.tensor.matmul(y5_ps[...], wct[:, kc, :], y2l[:, kc:kc + 1],
                         start=(kc == 0), stop=(kc == 1))
    y5 = wp.tile([40, 1], F32, name="y5", tag="y5")
    nc.scalar.activation(y5[...], y5_ps[...], AF.Identity, bias=cout[...])
    nc.sync.dma_start(io["out"], y5[...])

    ctx.close()


def _install_profile_hook():
    """The agent image's antenv lacks axon_hooks; recreate it so trace=True
    can drive NTFF profiling through libaxon_pjrt.so (test-only path)."""
    import types
    try:
        from antenv.axon_hooks import get_axon_ntff_profile_hook  # noqa: F401
        return
    except ImportError:
        pass
    mod = types.ModuleType("antenv.axon_hooks")
    _h = [None]
    mod.set_axon_ntff_profile_hook = lambda h: _h.__setitem__(0, h)
    mod.get_axon_ntff_profile_hook = lambda: _h[0]
    import antenv
    antenv.axon_hooks = mod
    sys.modules["antenv.axon_hooks"] = mod
    if "/root/.axon_site" not in sys.path:
        sys.path.insert(0, "/root/.axon_site")
    from trn_agent_boot.trn_boot import _ntff_profile_via_ctypes
    mod.set_axon_ntff_profile_hook(
        _ntff_profile_via_ctypes("/opt/axon/libaxon_pjrt.so"))
    # artifact upload needs the fish bucket; stub it out in-container
    import concourse.bass_utils as _bu
    _bu.upload_artifacts = lambda tmpdir: tmpdir


# --------------------------------------------------------------- build + run
_CACHE = {}


def _build_program(const_shapes):
    nc = bacc.Bacc("TRN2", target_bir_lowering=False, debug=False,
                   enable_asserts=False, num_devices=B)
    io = {}
    io["hx"] = nc.dram_tensor("hx", [3, N], F32, kind="ExternalInput").ap()
    for name, shp in const_shapes.items():
        io[name] = nc.dram_tensor(name, list(shp), F32, kind="ExternalInput").ap()
    io["out"] = nc.dram_tensor("out", [40], F32, kind="ExternalOutput").ap()
    with tile.TileContext(nc) as tc:
        _emit(tc, io)
    nc.compile()
    return nc


def kernel(**inputs):
    consts = _build_consts(inputs)
    key = "prog"
    if key not in _CACHE:
        _CACHE[key] = _build_program({k: v.shape for k, v in consts.items()})
    nc = _CACHE[key]

    x = np.asarray(inputs["x"], np.float32)
    in_maps = []
    for bi in range(B):
        m = {"hx": np.ascontiguousarray(x[bi])}
        m.update(consts)
        in_maps.append(m)

    trace = bool(int(os.environ.get("KERNEL_TRACE", "0")))
    if trace:
        _install_profile_hook()
    res = run_bass_kernel_spmd(nc, in_maps, core_ids=list(range(B)), trace=trace)
    kernel.last_result = res
    out = np.stack([r["out"] for r in res.results], axis=0).astype(np.float32)
    return out


if __name__ == "__main__":
    import reference as R
    inp = {k: np.asarray(v) for k, v in R.setup_inputs().items()}
    got = kernel(**inp)
    exp = np.asarray(R.reference(**R.setup_inputs()))
    err = np.abs(got - exp).max() / np.abs(exp).max()
    print("rel err:", err)


# revision 13
# speedup vs baseline: 2.9195x; 1.0193x over previous
"""DGCNN classification kernel for 8x Trainium2 NeuronCores.

Data-parallel: one point cloud (N=1024 points) per core, 8 clouds total.

Algorithmic restructuring vs the reference (numerically equivalent):
  EdgeConv: max_k LeakyReLU(BN(W @ [h_j; h_i]))
          = LeakyReLU( max_{j in knn(i)} (A1 @ h_j)  +  (A2 @ h_i + c) )
  where A = diag(bn_scale) @ W (split A1|A2 over the nbr/ctr halves) and
  c folds the BN shift; LeakyReLU commutes with max (monotone).
  This removes the [N,k,2C] edge tensor entirely: per layer we need
    - pairwise scores  s = h h^T - 0.5*||h_j||^2   (ranking-equiv to -dist^2)
    - top-20 per row (DVE max8/max_index/match_replace, 3 rounds)
    - U = h @ A1^T rows gathered by index (SWDGE dma_gather) + max over k
    - V = h @ A2^T + c, out = leaky(maxU + V)
  The trailing l3/l4/l5 affine chain is collapsed on the host into one
  [40 x 256] affine.

dma_gather wants indices wrapped [128, n/16] int16 (each Q7 core reads its
own 16 partitions; flat order J[s*16+p] with J[t*128+i] = knn[i][t] so the
gathered row for (i, t) lands at out[i, t, :]). The wrapped layout is built
on-chip with PE transposes + a replication matmul.
"""

import os
import sys

import numpy as np

sys.path.insert(0, "/opt/trn_rl_repo")

from contextlib import ExitStack  # noqa: E402

import concourse.bacc as bacc  # noqa: E402
import concourse.mybir as mybir  # noqa: E402
import concourse.tile as tile  # noqa: E402
from concourse.bass_utils import run_bass_kernel_spmd  # noqa: E402

F32 = mybir.dt.float32
U16 = mybir.dt.uint16
I16 = mybir.dt.int16
AF = mybir.ActivationFunctionType
ALU = mybir.AluOpType
AX = mybir.AxisListType

N = 1024
K = 20
B = 8
EPS = 1e-5
NEG = -3.0e38
LAYERS = [(3, 64), (64, 64), (64, 128), (128, 256)]  # (C_in, C_out)
NT = N // 128  # 8 m-tiles


# ----------------------------------------------------------------- host math
def _fold_bn(w, bn):
    g, b, m, v = [np.asarray(x, np.float32) for x in bn]
    s = (g * (1.0 / np.sqrt(v + EPS))).astype(np.float32)
    A = (s[:, None] * np.asarray(w, np.float32)).astype(np.float32)
    c = (b - m * s).astype(np.float32)
    return A, c


def _build_consts(inp):
    """All per-problem constants, shared by every core. Returns name->array."""
    c = {}
    for li, wkey, bkey in [(0, "w1", "bn1"), (1, "w2", "bn2"),
                           (2, "w3", "bn3"), (3, "w4", "bn4")]:
        Cin, Cout = LAYERS[li]
        A, cc = _fold_bn(inp[wkey], inp[bkey])
        A1, A2 = A[:, :Cin], A[:, Cin:]
        c[f"a1t{li}"] = np.ascontiguousarray(A1.T)            # [Cin, Cout]
        c[f"a2t{li}"] = np.ascontiguousarray(A2.T)            # [Cin, Cout]
        c[f"crow{li}"] = cc.reshape(1, Cout).copy()           # [1, Cout]
    # conv5
    A5, c5 = _fold_bn(inp["w5"], inp["bn5"])                  # [128, 512]
    ofs = [0, 64, 128, 256, 384, 512]
    for j in range(5):
        c[f"a5t{j}"] = np.ascontiguousarray(A5[:, ofs[j]:ofs[j + 1]].T)
    c["c5col"] = c5.reshape(128, 1).copy()
    # classifier layer 1 (512 <- 256), BN6 + leaky
    A6, c6 = _fold_bn(inp["l1w"], inp["bn6"])                 # [512, 256]
    c["a6at"] = np.ascontiguousarray(A6[:, :128].T)           # [128, 512]
    c["a6bt"] = np.ascontiguousarray((A6[:, 128:] / 1024.0).T)  # [128, 512]
    c["c6"] = np.ascontiguousarray(c6.reshape(4, 128).T)      # [128, 4]
    # classifier layer 2 (256 <- 512), +l2b then BN7 + leaky
    A7, c7 = _fold_bn(inp["l2w"], inp["bn7"])                 # [256, 512]
    s7 = np.asarray(inp["bn7"], np.float32)
    gs7 = (s7[0] * (1.0 / np.sqrt(s7[3] + EPS))).astype(np.float32)
    c7 = (c7 + gs7 * np.asarray(inp["l2b"], np.float32)).astype(np.float32)
    # lhsT layout: a7t[p, kc, m] = A7[m, kc*128+p]
    c["a7t"] = np.ascontiguousarray(A7.T.reshape(4, 128, 256).transpose(1, 0, 2))
    c["c7"] = np.ascontiguousarray(c7.reshape(2, 128).T)      # [128, 2]
    # collapse l3/l4/l5 into one affine [40, 256]
    l3w = np.asarray(inp["l3w"], np.float32); l3b = np.asarray(inp["l3b"], np.float32)
    l4w = np.asarray(inp["l4w"], np.float32); l4b = np.asarray(inp["l4b"], np.float32)
    l5w = np.asarray(inp["l5w"], np.float32); l5b = np.asarray(inp["l5b"], np.float32)
    Wc = (l5w @ l4w @ l3w).astype(np.float32)                 # [40, 256]
    bc = (l5w @ (l4w @ l3b + l4b) + l5b).astype(np.float32)   # [40]
    c["wct"] = np.ascontiguousarray(Wc.T.reshape(2, 128, 40).transpose(1, 0, 2))
    c["cout"] = bc.reshape(40, 1).copy()                      # [40, 1]
    c["ident"] = np.eye(128, dtype=np.float32)
    idrep = np.zeros((16, 128), np.float32)                   # replicate %16
    idrep[np.arange(128) % 16, np.arange(128)] = 1.0
    c["idrep"] = idrep
    return c


# --------------------------------------------------------------- the program
def _emit(tc, io):
    """Emit the full per-core program. io: name -> DRAM AP."""
    nc = tc.nc
    ctx = ExitStack()

    cp = ctx.enter_context(tc.tile_pool(name="consts", bufs=1))
    hp = ctx.enter_context(tc.tile_pool(name="hbufs", bufs=1))
    sp = ctx.enter_context(tc.tile_pool(name="swork", bufs=3))
    wp = ctx.enter_context(tc.tile_pool(name="work", bufs=2))
    ip = ctx.enter_context(tc.tile_pool(name="idxp", bufs=2))
    vp = ctx.enter_context(tc.tile_pool(name="needles", bufs=3))
    nb = ctx.enter_context(tc.tile_pool(name="nbrp", bufs=2))
    dp = ctx.enter_context(tc.tile_pool(name="dramp", bufs=1, space="DRAM"))
    ps_s = ctx.enter_context(tc.tile_pool(name="ps_s", bufs=2, space="PSUM"))
    ps_u = ctx.enter_context(tc.tile_pool(name="ps_u", bufs=1, space="PSUM"))
    ps_v = ctx.enter_context(tc.tile_pool(name="ps_v", bufs=2, space="PSUM"))
    ps_a = ctx.enter_context(tc.tile_pool(name="ps_a", bufs=1, space="PSUM"))
    ps_r = ctx.enter_context(tc.tile_pool(name="ps_r", bufs=2, space="PSUM"))

    def load_const(name):
        shp = list(io[name].shape)
        t = cp.tile(shp, F32, name=f"c_{name}", tag=f"c_{name}")
        nc.sync.dma_start(t[...], io[name])
        return t

    consts = {k: load_const(k) for k in io if k not in ("hx", "out")}
    ones_row = cp.tile([1, 128], F32, name="ones_row", tag="ones_row")
    nc.gpsimd.memset(ones_row[:], 1.0)
    neghalf = cp.tile([128, 1], F32, name="neghalf", tag="neghalf")
    nc.gpsimd.memset(neghalf[:], -0.5)
    ident = consts["ident"]
    idrep = consts["idrep"]

    # h^T buffers, feature-major [C, N]
    hxT = hp.tile([3, N], F32, name="hxT", tag="hxT")
    nc.sync.dma_start(hxT[...], io["hx"])
    h1T = hp.tile([64, N], F32, name="h1T", tag="h1T")
    h2T = hp.tile([64, N], F32, name="h2T", tag="h2T")
    h3T = hp.tile([128, N], F32, name="h3T", tag="h3T")
    h4Ta = hp.tile([128, N], F32, name="h4Ta", tag="h4Ta")
    h4Tb = hp.tile([128, N], F32, name="h4Tb", tag="h4Tb")
    h5T = hp.tile([128, N], F32, name="h5T", tag="h5T")

    h_in = [hxT, h1T, h2T, h3T]
    h_out = [[h1T], [h2T], [h3T], [h4Ta, h4Tb]]

    # ------------------------------------------------------------ edge convs
    for li, (Cin, Cout) in enumerate(LAYERS):
        hT = h_in[li]
        a1t = consts[f"a1t{li}"]
        a2t = consts[f"a2t{li}"]
        crow = consts[f"crow{li}"]
        u_dram = dp.tile([N, Cout], F32, name=f"u_dram{li}", tag=f"u_dram{li}")

        # squared norms: xx[n] = -0.5 * sum_c h[c,n]^2, via Square + ones-matmul
        sq = wp.tile([Cin, N], F32, name=f"sq{li}", tag="sq")
        nc.scalar.activation(sq[...], hT[...], AF.Square)
        xx_sb = wp.tile([1, N], F32, name=f"xx{li}", tag="xx")
        for nt2 in range(2):
            xx_ps = ps_a.tile([1, 512], F32, name=f"xxps{li}_{nt2}", tag="aux")
            nc.tensor.matmul(xx_ps[...], neghalf[0:Cin, :],
                             sq[:, nt2 * 512:(nt2 + 1) * 512], start=True, stop=True)
            nc.scalar.copy(xx_sb[:, nt2 * 512:(nt2 + 1) * 512], xx_ps[...])

        # ---------------- phase A per m-tile: scores, topk, U
        idx_all = ip.tile([128, NT, 24], U16, name=f"idxall{li}", tag="idxall")
        idxs_all = ip.tile([128, NT, 160], I16, name=f"idxsall{li}", tag="idxsall")
        for m in range(NT):
            mb = slice(m * 128, (m + 1) * 128)
            s_sb = sp.tile([128, N], F32, name=f"s{li}_{m}", tag="s_sb")
            for nt2 in range(2):
                ns = slice(nt2 * 512, (nt2 + 1) * 512)
                s_ps = ps_s.tile([128, 512], F32, name=f"sps{li}_{m}_{nt2}", tag="sps")
                nc.tensor.matmul(s_ps[...], hT[:, mb], hT[:, ns],
                                 start=True, stop=False)
                nc.tensor.matmul(s_ps[...], ones_row[...], xx_sb[:, ns],
                                 start=False, stop=True)
                nc.scalar.copy(s_sb[:, ns], s_ps[...])

            # top-20 of each row: 3 rounds of (max8, max_index) + match_replace
            for r in range(3):
                v8 = vp.tile([128, 8], F32, name=f"v8_{li}_{m}_{r}", tag="v8")
                nc.vector.max(v8[...], s_sb[...])
                nc.vector.max_index(idx_all[:, m, r * 8:(r + 1) * 8], v8[...],
                                    s_sb[...])
                if r < 2:
                    nc.vector.match_replace(s_sb[...], v8[...], s_sb[...], NEG)

            # U tile: [128 pts, Cout] point-major, stored to DRAM for the gather
            u_ps = ps_u.tile([128, Cout], F32, name=f"ups{li}_{m}", tag="ups")
            nc.tensor.matmul(u_ps[...], hT[:, mb], a1t[...], start=True, stop=True)
            u_sb = wp.tile([128, Cout], F32, name=f"usb{li}_{m}", tag="usb")
            nc.scalar.copy(u_sb[...], u_ps[...])
            nc.sync.dma_start(u_dram[mb, :], u_sb[...])

            if m % 4 == 3:
                # wrapped idx-layout build for this half (m-3 .. m)
                h0 = m - 3
                idxf_h = wp.tile([128, 4, 20], F32, name=f"idxf{li}_{h0}",
                                 tag="idxfall")
                nc.vector.tensor_copy(idxf_h[...], idx_all[:, h0:h0 + 4, 0:20])
                for g in range(2):          # rep groups of 2 m-tiles
                    rep_ps = ps_r.tile([128, 320], F32,
                                       name=f"repps{li}_{h0}_{g}", tag="repall")
                    for j in range(2):
                        mm = h0 + g * 2 + j
                        mt_ps = ps_a.tile([20, 128], F32, name=f"mtps{li}_{mm}",
                                          tag="aux")
                        nc.tensor.transpose(mt_ps[...], idxf_h[:, g * 2 + j, :],
                                            ident[...])
                        mt_sb = wp.tile([20, 128], F32, name=f"mtsb{li}_{mm}",
                                        tag="mtsb")
                        nc.scalar.copy(mt_sb[...], mt_ps[...])
                        tball = ps_a.tile([16, 20, 8], F32,
                                          name=f"tball{li}_{mm}", tag="aux")
                        for bb in range(8):
                            nc.tensor.transpose(tball[:, :, bb],
                                                mt_sb[:, bb * 16:(bb + 1) * 16],
                                                ident[0:20, 0:20])
                        wsb = wp.tile([16, 160], F32, name=f"wsb{li}_{mm}",
                                      tag="wsb")
                        nc.scalar.copy(wsb[...],
                                       tball[...].rearrange("p t b -> p (t b)"))
                        nc.tensor.matmul(rep_ps[:, j * 160:(j + 1) * 160],
                                         idrep[...], wsb[...],
                                         start=True, stop=True)
                    nc.vector.tensor_copy(
                        idxs_all[:, h0 + g * 2:h0 + g * 2 + 2, :]
                        .rearrange("p a b -> p (a b)"),
                        rep_ps[...])

        # (idx layout build happens per-half inside the loop above)

        # ---------------- phase B per m-tile: gather, k-max, V, combine
        for m in range(NT):
            mb = slice(m * 128, (m + 1) * 128)
            nbr = nb.tile([128, K, Cout], F32, name=f"nbr{li}_{m}", tag="nbr")
            nc.gpsimd.dma_gather(nbr[...], u_dram[...], idxs_all[:, m, :],
                                 num_idxs=128 * K, num_idxs_reg=128 * K,
                                 elem_size=Cout, single_packet=False)
            mx = wp.tile([128, Cout], F32, name=f"mx{li}_{m}", tag="mx")
            nc.vector.tensor_reduce(mx[...], nbr[...].rearrange("p t c -> p c t"),
                                    axis=AX.X, op=ALU.max)

            v_ps = ps_v.tile([128, Cout], F32, name=f"vps{li}_{m}", tag="vps")
            nc.tensor.matmul(v_ps[...], hT[:, mb], a2t[...], start=True, stop=False)
            nc.tensor.matmul(v_ps[...], ones_row[...], crow[...],
                             start=False, stop=True)

            z = wp.tile([128, Cout], F32, name=f"z{li}_{m}", tag="z")
            nc.vector.tensor_tensor(z[...], mx[...], v_ps[...], ALU.add)
            hpm = wp.tile([128, Cout], F32, name=f"hpm{li}_{m}", tag="hpm")
            nc.vector.scalar_tensor_tensor(hpm[...], z[...], 0.2, z[...],
                                           op0=ALU.mult, op1=ALU.max)

            # transpose back to feature-major into next layer's h^T buffer
            for ci, hdst in enumerate(h_out[li]):
                cs = slice(ci * 128, min((ci + 1) * 128, Cout))
                w = cs.stop - cs.start
                t_ps = ps_a.tile([w, 128], F32, name=f"tps{li}_{m}_{ci}", tag="aux")
                nc.tensor.transpose(t_ps[...], hpm[:, cs], ident[...])
                nc.scalar.copy(hdst[0:w, mb], t_ps[...])

    # ------------------------------------------------------------ conv5
    a5 = [consts[f"a5t{j}"] for j in range(5)]
    srcs = [h1T, h2T, h3T, h4Ta, h4Tb]
    for nt2 in range(2):
        ns = slice(nt2 * 512, (nt2 + 1) * 512)
        h5_ps = ps_s.tile([128, 512], F32, name=f"h5ps{nt2}", tag="sps")
        for j in range(5):
            nc.tensor.matmul(h5_ps[...], a5[j][...], srcs[j][:, ns],
                             start=(j == 0), stop=(j == 4))
        # h5 = leaky(h5_ps + c5); c5 is per-channel = per-partition scalar bias
        zt = sp.tile([128, 512], F32, name=f"h5z{nt2}", tag="s_sb")
        nc.scalar.activation(zt[:, 0:512], h5_ps[...], AF.Identity,
                             bias=consts["c5col"][...])
        nc.vector.scalar_tensor_tensor(h5T[:, ns], zt[:, 0:512], 0.2,
                                       zt[:, 0:512], op0=ALU.mult, op1=ALU.max)

    # ------------------------------------------------------------ pooling
    gmax = wp.tile([128, 1], F32, name="gmax", tag="gpool")
    nc.vector.tensor_reduce(gmax[...], h5T[...], axis=AX.X, op=ALU.max)
    gsum = wp.tile([128, 1], F32, name="gsum", tag="gpool")
    nc.vector.tensor_reduce(gsum[...], h5T[...], axis=AX.X, op=ALU.add)

    # ------------------------------------------------------------ classifier
    a6at, a6bt, c6 = consts["a6at"], consts["a6bt"], consts["c6"]
    y1 = wp.tile([128, 4], F32, name="y1", tag="y1")
    for mt in range(4):
        ms = slice(mt * 128, (mt + 1) * 128)
        y_ps = ps_v.tile([128, 1], F32, name=f"y1ps{mt}", tag="vps")
        nc.tensor.matmul(y_ps[...], a6at[:, ms], gmax[...], start=True, stop=False)
        nc.tensor.matmul(y_ps[...], a6bt[:, ms], gsum[...], start=False, stop=True)
        nc.scalar.activation(y1[:, mt:mt + 1], y_ps[...], AF.Identity,
                             bias=c6[:, mt:mt + 1])
    y1l = wp.tile([128, 4], F32, name="y1l", tag="y1")
    nc.vector.scalar_tensor_tensor(y1l[...], y1[...], 0.2, y1[...],
                                   op0=ALU.mult, op1=ALU.max)

    a7t, c7 = consts["a7t"], consts["c7"]
    y2 = wp.tile([128, 2], F32, name="y2", tag="y2")
    for m2 in range(2):
        ms = slice(m2 * 128, (m2 + 1) * 128)
        y_ps = ps_v.tile([128, 1], F32, name=f"y2ps{m2}", tag="vps")
        for kc in range(4):
            nc.tensor.matmul(y_ps[...], a7t[:, kc, ms], y1l[:, kc:kc + 1],
                             start=(kc == 0), stop=(kc == 3))
        nc.scalar.activation(y2[:, m2:m2 + 1], y_ps[...], AF.Identity,
                             bias=c7[:, m2:m2 + 1])
    y2l = wp.tile([128, 2], F32, name="y2l", tag="y2")
    nc.vector.scalar_tensor_tensor(y2l[...], y2[...], 0.2, y2[...],
                                   op0=ALU.mult, op1=ALU.max)

    wct, cout = consts["wct"], consts["cout"]
    y5_ps = ps_v.tile([40, 1], F32, name="y5ps", tag="vps")
    for kc in range(2):
        nc.tensor.matmul(y5_ps[...], wct[:, kc, :], y2l[:, kc:kc + 1],
                         start=(kc == 0), stop=(kc == 1))
    y5 = wp.tile([40, 1], F32, name="y5", tag="y5")
    nc.scalar.activation(y5[...], y5_ps[...], AF.Identity, bias=cout[...])
    nc.sync.dma_start(io["out"], y5[...])

    ctx.close()


def _install_profile_hook():
    """The agent image's antenv lacks axon_hooks; recreate it so trace=True
    can drive NTFF profiling through libaxon_pjrt.so (test-only path)."""
    import types
    try:
        from antenv.axon_hooks import get_axon_ntff_profile_hook  # noqa: F401
        return
    except ImportError:
        pass
    mod = types.ModuleType("antenv.axon_hooks")
    _h = [None]
    mod.set_axon_ntff_profile_hook = lambda h: _h.__setitem__(0, h)
    mod.get_axon_ntff_profile_hook = lambda: _h[0]
    import antenv
    antenv.axon_hooks = mod
    sys.modules["antenv.axon_hooks"] = mod
    if "/root/.axon_site" not in sys.path:
        sys.path.insert(0, "/root/.axon_site")
    from trn_agent_boot.trn_boot import _ntff_profile_via_ctypes
    mod.set_axon_ntff_profile_hook(
        _ntff_profile_via_ctypes("/opt/axon/libaxon_pjrt.so"))
    # artifact upload needs the fish bucket; stub it out in-container
    import concourse.bass_utils as _bu
    _bu.upload_artifacts = lambda tmpdir: tmpdir


# --------------------------------------------------------------- build + run
_CACHE = {}


def _build_program(const_shapes):
    nc = bacc.Bacc("TRN2", target_bir_lowering=False, debug=False,
                   enable_asserts=False, num_devices=B)
    io = {}
    io["hx"] = nc.dram_tensor("hx", [3, N], F32, kind="ExternalInput").ap()
    for name, shp in const_shapes.items():
        io[name] = nc.dram_tensor(name, list(shp), F32, kind="ExternalInput").ap()
    io["out"] = nc.dram_tensor("out", [40], F32, kind="ExternalOutput").ap()
    with tile.TileContext(nc) as tc:
        _emit(tc, io)
    nc.compile()
    return nc


def kernel(**inputs):
    consts = _build_consts(inputs)
    key = "prog"
    if key not in _CACHE:
        _CACHE[key] = _build_program({k: v.shape for k, v in consts.items()})
    nc = _CACHE[key]

    x = np.asarray(inputs["x"], np.float32)
    in_maps = []
    for bi in range(B):
        m = {"hx": np.ascontiguousarray(x[bi])}
        m.update(consts)
        in_maps.append(m)

    trace = bool(int(os.environ.get("KERNEL_TRACE", "0")))
    if trace:
        _install_profile_hook()
    res = run_bass_kernel_spmd(nc, in_maps, core_ids=list(range(B)), trace=trace)
    kernel.last_result = res
    out = np.stack([r["out"] for r in res.results], axis=0).astype(np.float32)
    return out


if __name__ == "__main__":
    import reference as R
    inp = {k: np.asarray(v) for k, v in R.setup_inputs().items()}
    got = kernel(**inp)
    exp = np.asarray(R.reference(**R.setup_inputs()))
    err = np.abs(got - exp).max() / np.abs(exp).max()
    print("rel err:", err)


# revision 14
# speedup vs baseline: 3.2594x; 1.1164x over previous
"""DGCNN classification kernel for 8x Trainium2 NeuronCores.

Data-parallel: one point cloud (N=1024 points) per core, 8 clouds total.

Algorithmic restructuring vs the reference (numerically equivalent):
  EdgeConv: max_k LeakyReLU(BN(W @ [h_j; h_i]))
          = LeakyReLU( max_{j in knn(i)} (A1 @ h_j)  +  (A2 @ h_i + c) )
  where A = diag(bn_scale) @ W (split A1|A2 over the nbr/ctr halves) and
  c folds the BN shift; LeakyReLU commutes with max (monotone).
  This removes the [N,k,2C] edge tensor entirely: per layer we need
    - pairwise scores  s = h h^T - 0.5*||h_j||^2   (ranking-equiv to -dist^2)
    - top-20 per row (DVE max8/max_index/match_replace, 3 rounds)
    - U = h @ A1^T rows gathered by index (SWDGE dma_gather) + max over k
    - V = h @ A2^T + c, out = leaky(maxU + V)
  The trailing l3/l4/l5 affine chain is collapsed on the host into one
  [40 x 256] affine.

dma_gather wants indices wrapped [128, n/16] int16 (each Q7 core reads its
own 16 partitions; flat order J[s*16+p] with J[t*128+i] = knn[i][t] so the
gathered row for (i, t) lands at out[i, t, :]). The wrapped layout is built
on-chip with PE transposes + a replication matmul.
"""

import os
import sys

import numpy as np

sys.path.insert(0, "/opt/trn_rl_repo")

from contextlib import ExitStack  # noqa: E402

import concourse.bacc as bacc  # noqa: E402
import concourse.mybir as mybir  # noqa: E402
import concourse.tile as tile  # noqa: E402
from concourse.bass_utils import run_bass_kernel_spmd  # noqa: E402

F32 = mybir.dt.float32
U16 = mybir.dt.uint16
I16 = mybir.dt.int16
AF = mybir.ActivationFunctionType
ALU = mybir.AluOpType
AX = mybir.AxisListType

N = 1024
K = 20
B = 8
EPS = 1e-5
NEG = -3.0e38
LAYERS = [(3, 64), (64, 64), (64, 128), (128, 256)]  # (C_in, C_out)
NT = N // 128  # 8 m-tiles


# ----------------------------------------------------------------- host math
def _fold_bn(w, bn):
    g, b, m, v = [np.asarray(x, np.float32) for x in bn]
    s = (g * (1.0 / np.sqrt(v + EPS))).astype(np.float32)
    A = (s[:, None] * np.asarray(w, np.float32)).astype(np.float32)
    c = (b - m * s).astype(np.float32)
    return A, c


def _build_consts(inp):
    """All per-problem constants, shared by every core. Returns name->array."""
    c = {}
    for li, wkey, bkey in [(0, "w1", "bn1"), (1, "w2", "bn2"),
                           (2, "w3", "bn3"), (3, "w4", "bn4")]:
        Cin, Cout = LAYERS[li]
        A, cc = _fold_bn(inp[wkey], inp[bkey])
        A1, A2 = A[:, :Cin], A[:, Cin:]
        c[f"a1t{li}"] = np.ascontiguousarray(A1.T)            # [Cin, Cout]
        c[f"a2t{li}"] = np.ascontiguousarray(A2.T)            # [Cin, Cout]
        c[f"crow{li}"] = cc.reshape(1, Cout).copy()           # [1, Cout]
    # conv5
    A5, c5 = _fold_bn(inp["w5"], inp["bn5"])                  # [128, 512]
    ofs = [0, 64, 128, 256, 384, 512]
    for j in range(5):
        c[f"a5t{j}"] = np.ascontiguousarray(A5[:, ofs[j]:ofs[j + 1]].T)
    c["c5col"] = c5.reshape(128, 1).copy()
    # classifier layer 1 (512 <- 256), BN6 + leaky
    A6, c6 = _fold_bn(inp["l1w"], inp["bn6"])                 # [512, 256]
    c["a6at"] = np.ascontiguousarray(A6[:, :128].T)           # [128, 512]
    c["a6bt"] = np.ascontiguousarray((A6[:, 128:] / 1024.0).T)  # [128, 512]
    c["c6"] = np.ascontiguousarray(c6.reshape(4, 128).T)      # [128, 4]
    # classifier layer 2 (256 <- 512), +l2b then BN7 + leaky
    A7, c7 = _fold_bn(inp["l2w"], inp["bn7"])                 # [256, 512]
    s7 = np.asarray(inp["bn7"], np.float32)
    gs7 = (s7[0] * (1.0 / np.sqrt(s7[3] + EPS))).astype(np.float32)
    c7 = (c7 + gs7 * np.asarray(inp["l2b"], np.float32)).astype(np.float32)
    # lhsT layout: a7t[p, kc, m] = A7[m, kc*128+p]
    c["a7t"] = np.ascontiguousarray(A7.T.reshape(4, 128, 256).transpose(1, 0, 2))
    c["c7"] = np.ascontiguousarray(c7.reshape(2, 128).T)      # [128, 2]
    # collapse l3/l4/l5 into one affine [40, 256]
    l3w = np.asarray(inp["l3w"], np.float32); l3b = np.asarray(inp["l3b"], np.float32)
    l4w = np.asarray(inp["l4w"], np.float32); l4b = np.asarray(inp["l4b"], np.float32)
    l5w = np.asarray(inp["l5w"], np.float32); l5b = np.asarray(inp["l5b"], np.float32)
    Wc = (l5w @ l4w @ l3w).astype(np.float32)                 # [40, 256]
    bc = (l5w @ (l4w @ l3b + l4b) + l5b).astype(np.float32)   # [40]
    c["wct"] = np.ascontiguousarray(Wc.T.reshape(2, 128, 40).transpose(1, 0, 2))
    c["cout"] = bc.reshape(40, 1).copy()                      # [40, 1]
    c["ident"] = np.eye(128, dtype=np.float32)
    idrep = np.zeros((16, 128), np.float32)                   # replicate %16
    idrep[np.arange(128) % 16, np.arange(128)] = 1.0
    c["idrep"] = idrep
    return c


# --------------------------------------------------------------- the program
def _emit(tc, io):
    """Emit the full per-core program. io: name -> DRAM AP."""
    nc = tc.nc
    ctx = ExitStack()

    cp = ctx.enter_context(tc.tile_pool(name="consts", bufs=1))
    hp = ctx.enter_context(tc.tile_pool(name="hbufs", bufs=1))
    sp = ctx.enter_context(tc.tile_pool(name="swork", bufs=3))
    wp = ctx.enter_context(tc.tile_pool(name="work", bufs=2))
    ip = ctx.enter_context(tc.tile_pool(name="idxp", bufs=2))
    vp = ctx.enter_context(tc.tile_pool(name="needles", bufs=3))
    nb = ctx.enter_context(tc.tile_pool(name="nbrp", bufs=2))
    dp = ctx.enter_context(tc.tile_pool(name="dramp", bufs=1, space="DRAM"))
    ps_s = ctx.enter_context(tc.tile_pool(name="ps_s", bufs=2, space="PSUM"))
    ps_u = ctx.enter_context(tc.tile_pool(name="ps_u", bufs=1, space="PSUM"))
    ps_v = ctx.enter_context(tc.tile_pool(name="ps_v", bufs=2, space="PSUM"))
    ps_a = ctx.enter_context(tc.tile_pool(name="ps_a", bufs=1, space="PSUM"))
    ps_r = ctx.enter_context(tc.tile_pool(name="ps_r", bufs=2, space="PSUM"))

    def load_const(name):
        shp = list(io[name].shape)
        t = cp.tile(shp, F32, name=f"c_{name}", tag=f"c_{name}")
        nc.sync.dma_start(t[...], io[name])
        return t

    consts = {k: load_const(k) for k in io if k not in ("hx", "out")}
    ones_row = cp.tile([1, 128], F32, name="ones_row", tag="ones_row")
    nc.gpsimd.memset(ones_row[:], 1.0)
    neghalf = cp.tile([128, 1], F32, name="neghalf", tag="neghalf")
    nc.gpsimd.memset(neghalf[:], -0.5)
    ident = consts["ident"]
    idrep = consts["idrep"]

    # h^T buffers, feature-major [C, N]
    hxT = hp.tile([3, N], F32, name="hxT", tag="hxT")
    nc.sync.dma_start(hxT[...], io["hx"])
    h1T = hp.tile([64, N], F32, name="h1T", tag="h1T")
    h2T = hp.tile([64, N], F32, name="h2T", tag="h2T")
    h3T = hp.tile([128, N], F32, name="h3T", tag="h3T")
    h4Ta = hp.tile([128, N], F32, name="h4Ta", tag="h4Ta")
    h4Tb = hp.tile([128, N], F32, name="h4Tb", tag="h4Tb")
    h5T = hp.tile([128, N], F32, name="h5T", tag="h5T")

    h_in = [hxT, h1T, h2T, h3T]
    h_out = [[h1T], [h2T], [h3T], [h4Ta, h4Tb]]

    # ------------------------------------------------------------ edge convs
    for li, (Cin, Cout) in enumerate(LAYERS):
        hT = h_in[li]
        a1t = consts[f"a1t{li}"]
        a2t = consts[f"a2t{li}"]
        crow = consts[f"crow{li}"]
        u_dram = dp.tile([N, Cout], F32, name=f"u_dram{li}", tag=f"u_dram{li}")

        # squared norms: xx[n] = -0.5 * sum_c h[c,n]^2, via Square + ones-matmul
        sq = wp.tile([Cin, N], F32, name=f"sq{li}", tag="sq")
        nc.scalar.activation(sq[...], hT[...], AF.Square)
        xx_sb = wp.tile([1, N], F32, name=f"xx{li}", tag="xx")
        for nt2 in range(2):
            xx_ps = ps_a.tile([1, 512], F32, name=f"xxps{li}_{nt2}", tag="aux")
            nc.tensor.matmul(xx_ps[...], neghalf[0:Cin, :],
                             sq[:, nt2 * 512:(nt2 + 1) * 512], start=True, stop=True)
            nc.scalar.copy(xx_sb[:, nt2 * 512:(nt2 + 1) * 512], xx_ps[...])

        # ---------------- phase A per m-tile: scores, topk, U
        idx_all = ip.tile([128, NT, 24], U16, name=f"idxall{li}", tag="idxall")
        idxs_all = ip.tile([128, NT, 160], I16, name=f"idxsall{li}", tag="idxsall")
        for m in range(NT):
            mb = slice(m * 128, (m + 1) * 128)
            s_sb = sp.tile([128, N], F32, name=f"s{li}_{m}", tag="s_sb")
            for nt2 in range(2):
                ns = slice(nt2 * 512, (nt2 + 1) * 512)
                s_ps = ps_s.tile([128, 512], F32, name=f"sps{li}_{m}_{nt2}", tag="sps")
                nc.tensor.matmul(s_ps[...], hT[:, mb], hT[:, ns],
                                 start=True, stop=False)
                nc.tensor.matmul(s_ps[...], ones_row[...], xx_sb[:, ns],
                                 start=False, stop=True)
                nc.scalar.copy(s_sb[:, ns], s_ps[...])

            # top-20 of each row: 3 rounds of (max8, max_index) + match_replace
            for r in range(3):
                v8 = vp.tile([128, 8], F32, name=f"v8_{li}_{m}_{r}", tag="v8")
                nc.vector.max(v8[...], s_sb[...])
                nc.vector.max_index(idx_all[:, m, r * 8:(r + 1) * 8], v8[...],
                                    s_sb[...])
                if r < 2:
                    nc.vector.match_replace(s_sb[...], v8[...], s_sb[...], NEG)

            # U tile: [128 pts, Cout] point-major, stored to DRAM for the gather
            u_ps = ps_u.tile([128, Cout], F32, name=f"ups{li}_{m}", tag="ups")
            nc.tensor.matmul(u_ps[...], hT[:, mb], a1t[...], start=True, stop=True)
            u_sb = wp.tile([128, Cout], F32, name=f"usb{li}_{m}", tag="usb")
            nc.scalar.copy(u_sb[...], u_ps[...])
            nc.sync.dma_start(u_dram[mb, :], u_sb[...])

            if m % 4 == 3:
                # wrapped idx-layout build for this half (m-3 .. m)
                h0 = m - 3
                idxf_h = wp.tile([128, 4, 20], F32, name=f"idxf{li}_{h0}",
                                 tag="idxfall")
                nc.vector.tensor_copy(idxf_h[...], idx_all[:, h0:h0 + 4, 0:20])
                for g in range(2):          # rep groups of 2 m-tiles
                    rep_ps = ps_r.tile([128, 320], F32,
                                       name=f"repps{li}_{h0}_{g}", tag="repall")
                    for j in range(2):
                        mm = h0 + g * 2 + j
                        mt_ps = ps_a.tile([20, 128], F32, name=f"mtps{li}_{mm}",
                                          tag="aux")
                        nc.tensor.transpose(mt_ps[...], idxf_h[:, g * 2 + j, :],
                                            ident[...])
                        mt_sb = wp.tile([20, 128], F32, name=f"mtsb{li}_{mm}",
                                        tag="mtsb")
                        nc.scalar.copy(mt_sb[...], mt_ps[...])
                        tball = ps_a.tile([16, 20, 8], F32,
                                          name=f"tball{li}_{mm}", tag="aux")
                        for bb in range(8):
                            nc.tensor.transpose(tball[:, :, bb],
                                                mt_sb[:, bb * 16:(bb + 1) * 16],
                                                ident[0:20, 0:20])
                        wsb = wp.tile([16, 160], F32, name=f"wsb{li}_{mm}",
                                      tag="wsb")
                        nc.scalar.copy(wsb[...],
                                       tball[...].rearrange("p t b -> p (t b)"))
                        nc.tensor.matmul(rep_ps[:, j * 160:(j + 1) * 160],
                                         idrep[...], wsb[...],
                                         start=True, stop=True)
                    nc.vector.tensor_copy(
                        idxs_all[:, h0 + g * 2:h0 + g * 2 + 2, :]
                        .rearrange("p a b -> p (a b)"),
                        rep_ps[...])

        # (idx layout build happens per-half inside the loop above)

        # ---------------- phase B per m-tile: gather, k-max, V, combine
        for m in range(NT):
            mb = slice(m * 128, (m + 1) * 128)
            nbr = nb.tile([128, K, Cout], F32, name=f"nbr{li}_{m}", tag="nbr")
            nc.gpsimd.dma_gather(nbr[...], u_dram[...], idxs_all[:, m, :],
                                 num_idxs=128 * K, num_idxs_reg=128 * K,
                                 elem_size=Cout, single_packet=False,
                                 queue_num=m % 4)
            mx = wp.tile([128, Cout], F32, name=f"mx{li}_{m}", tag="mx")
            nc.vector.tensor_reduce(mx[...], nbr[...].rearrange("p t c -> p c t"),
                                    axis=AX.X, op=ALU.max)

            v_ps = ps_v.tile([128, Cout], F32, name=f"vps{li}_{m}", tag="vps")
            nc.tensor.matmul(v_ps[...], hT[:, mb], a2t[...], start=True, stop=False)
            nc.tensor.matmul(v_ps[...], ones_row[...], crow[...],
                             start=False, stop=True)

            z = wp.tile([128, Cout], F32, name=f"z{li}_{m}", tag="z")
            nc.vector.tensor_tensor(z[...], mx[...], v_ps[...], ALU.add)
            hpm = wp.tile([128, Cout], F32, name=f"hpm{li}_{m}", tag="hpm")
            nc.vector.scalar_tensor_tensor(hpm[...], z[...], 0.2, z[...],
                                           op0=ALU.mult, op1=ALU.max)

            # transpose back to feature-major into next layer's h^T buffer
            for ci, hdst in enumerate(h_out[li]):
                cs = slice(ci * 128, min((ci + 1) * 128, Cout))
                w = cs.stop - cs.start
                t_ps = ps_a.tile([w, 128], F32, name=f"tps{li}_{m}_{ci}", tag="aux")
                nc.tensor.transpose(t_ps[...], hpm[:, cs], ident[...])
                nc.scalar.copy(hdst[0:w, mb], t_ps[...])

    # ------------------------------------------------------------ conv5
    a5 = [consts[f"a5t{j}"] for j in range(5)]
    srcs = [h1T, h2T, h3T, h4Ta, h4Tb]
    for nt2 in range(2):
        ns = slice(nt2 * 512, (nt2 + 1) * 512)
        h5_ps = ps_s.tile([128, 512], F32, name=f"h5ps{nt2}", tag="sps")
        for j in range(5):
            nc.tensor.matmul(h5_ps[...], a5[j][...], srcs[j][:, ns],
                             start=(j == 0), stop=(j == 4))
        # h5 = leaky(h5_ps + c5); c5 is per-channel = per-partition scalar bias
        zt = sp.tile([128, 512], F32, name=f"h5z{nt2}", tag="s_sb")
        nc.scalar.activation(zt[:, 0:512], h5_ps[...], AF.Identity,
                             bias=consts["c5col"][...])
        nc.vector.scalar_tensor_tensor(h5T[:, ns], zt[:, 0:512], 0.2,
                                       zt[:, 0:512], op0=ALU.mult, op1=ALU.max)

    # ------------------------------------------------------------ pooling
    gmax = wp.tile([128, 1], F32, name="gmax", tag="gpool")
    nc.vector.tensor_reduce(gmax[...], h5T[...], axis=AX.X, op=ALU.max)
    gsum = wp.tile([128, 1], F32, name="gsum", tag="gpool")
    nc.vector.tensor_reduce(gsum[...], h5T[...], axis=AX.X, op=ALU.add)

    # ------------------------------------------------------------ classifier
    a6at, a6bt, c6 = consts["a6at"], consts["a6bt"], consts["c6"]
    y1 = wp.tile([128, 4], F32, name="y1", tag="y1")
    for mt in range(4):
        ms = slice(mt * 128, (mt + 1) * 128)
        y_ps = ps_v.tile([128, 1], F32, name=f"y1ps{mt}", tag="vps")
        nc.tensor.matmul(y_ps[...], a6at[:, ms], gmax[...], start=True, stop=False)
        nc.tensor.matmul(y_ps[...], a6bt[:, ms], gsum[...], start=False, stop=True)
        nc.scalar.activation(y1[:, mt:mt + 1], y_ps[...], AF.Identity,
                             bias=c6[:, mt:mt + 1])
    y1l = wp.tile([128, 4], F32, name="y1l", tag="y1")
    nc.vector.scalar_tensor_tensor(y1l[...], y1[...], 0.2, y1[...],
                                   op0=ALU.mult, op1=ALU.max)

    a7t, c7 = consts["a7t"], consts["c7"]
    y2 = wp.tile([128, 2], F32, name="y2", tag="y2")
    for m2 in range(2):
        ms = slice(m2 * 128, (m2 + 1) * 128)
        y_ps = ps_v.tile([128, 1], F32, name=f"y2ps{m2}", tag="vps")
        for kc in range(4):
            nc.tensor.matmul(y_ps[...], a7t[:, kc, ms], y1l[:, kc:kc + 1],
                             start=(kc == 0), stop=(kc == 3))
        nc.scalar.activation(y2[:, m2:m2 + 1], y_ps[...], AF.Identity,
                             bias=c7[:, m2:m2 + 1])
    y2l = wp.tile([128, 2], F32, name="y2l", tag="y2")
    nc.vector.scalar_tensor_tensor(y2l[...], y2[...], 0.2, y2[...],
                                   op0=ALU.mult, op1=ALU.max)

    wct, cout = consts["wct"], consts["cout"]
    y5_ps = ps_v.tile([40, 1], F32, name="y5ps", tag="vps")
    for kc in range(2):
        nc.tensor.matmul(y5_ps[...], wct[:, kc, :], y2l[:, kc:kc + 1],
                         start=(kc == 0), stop=(kc == 1))
    y5 = wp.tile([40, 1], F32, name="y5", tag="y5")
    nc.scalar.activation(y5[...], y5_ps[...], AF.Identity, bias=cout[...])
    nc.sync.dma_start(io["out"], y5[...])

    ctx.close()


def _install_profile_hook():
    """The agent image's antenv lacks axon_hooks; recreate it so trace=True
    can drive NTFF profiling through libaxon_pjrt.so (test-only path)."""
    import types
    try:
        from antenv.axon_hooks import get_axon_ntff_profile_hook  # noqa: F401
        return
    except ImportError:
        pass
    mod = types.ModuleType("antenv.axon_hooks")
    _h = [None]
    mod.set_axon_ntff_profile_hook = lambda h: _h.__setitem__(0, h)
    mod.get_axon_ntff_profile_hook = lambda: _h[0]
    import antenv
    antenv.axon_hooks = mod
    sys.modules["antenv.axon_hooks"] = mod
    if "/root/.axon_site" not in sys.path:
        sys.path.insert(0, "/root/.axon_site")
    from trn_agent_boot.trn_boot import _ntff_profile_via_ctypes
    mod.set_axon_ntff_profile_hook(
        _ntff_profile_via_ctypes("/opt/axon/libaxon_pjrt.so"))
    # artifact upload needs the fish bucket; stub it out in-container
    import concourse.bass_utils as _bu
    _bu.upload_artifacts = lambda tmpdir: tmpdir


# --------------------------------------------------------------- build + run
_CACHE = {}


def _build_program(const_shapes):
    nc = bacc.Bacc("TRN2", target_bir_lowering=False, debug=False,
                   enable_asserts=False, num_devices=B, num_swdge_queues=4)
    io = {}
    io["hx"] = nc.dram_tensor("hx", [3, N], F32, kind="ExternalInput").ap()
    for name, shp in const_shapes.items():
        io[name] = nc.dram_tensor(name, list(shp), F32, kind="ExternalInput").ap()
    io["out"] = nc.dram_tensor("out", [40], F32, kind="ExternalOutput").ap()
    with tile.TileContext(nc) as tc:
        _emit(tc, io)
    nc.compile()
    return nc


def kernel(**inputs):
    consts = _build_consts(inputs)
    key = "prog"
    if key not in _CACHE:
        _CACHE[key] = _build_program({k: v.shape for k, v in consts.items()})
    nc = _CACHE[key]

    x = np.asarray(inputs["x"], np.float32)
    in_maps = []
    for bi in range(B):
        m = {"hx": np.ascontiguousarray(x[bi])}
        m.update(consts)
        in_maps.append(m)

    trace = bool(int(os.environ.get("KERNEL_TRACE", "0")))
    if trace:
        _install_profile_hook()
    res = run_bass_kernel_spmd(nc, in_maps, core_ids=list(range(B)), trace=trace)
    kernel.last_result = res
    out = np.stack([r["out"] for r in res.results], axis=0).astype(np.float32)
    return out


if __name__ == "__main__":
    import reference as R
    inp = {k: np.asarray(v) for k, v in R.setup_inputs().items()}
    got = kernel(**inp)
    exp = np.asarray(R.reference(**R.setup_inputs()))
    err = np.abs(got - exp).max() / np.abs(exp).max()
    print("rel err:", err)
